# revision 1
# baseline (speedup 1.0000x reference)
"""GuidedAttentionLoss on 8 Trainium2 NeuronCores.

Math: loss = mean_b( sum_{f<F_b, l<L_b} A[b,f,l] * w[b,f,l] / F_b ),
      w = 1 - exp(-c*(l/L - f/F)^2),  c = 1/(2*gamma^(2*step)).

Key identity: exp(-c(x-y)^2) = exp(-cx^2)*exp(-cy^2)*exp(2cxy), and
exp(z) on z in [0, 2c) is approximated by a degree-D polynomial, so the
Gaussian weight is separable:  e[f,l] = sum_k h_k[f] * g_k[l]  with
  h_k[f] = a_k * (2c*y)^k * exp(-c*y^2),  y = f/F   (k = 0..D)
  g_k[l] = x^k * exp(-c*x^2),             x = l/L.
Then sum_{f,l} A*e = sum_k sum_l g_k[l] * C[k,l] with
  C[k,l] = sum_f h_k[f] * A[f,l]   -- a tiny-M matmul Hc^T @ A
(an extra all-ones column of Hc gives sum_f A for the "1" term).

So the whole device kernel is: stream A row-chunks through the
TensorEngine against a small [128 x M] stationary weight, PSUM-
accumulating a [M x L] result per batch; the host does a tiny [M x L]
f64 epilogue. Pure DMA + matmul.

Precision: A is staged to the device as bf16 (element rounding is
unbiased and averages out over the 2048-row contraction; measured
~4e-6 on the final loss). The weights h are split hi/lo into two bf16
columns each (h = hi + lo), recovering ~fp32 weight precision at no
matmul cost (cost scales with N, not with the column count M).

Sharding: pure data parallel over batch. 64 batches -> 8 slots x 8
cores (SPMD: one program, per-core weights/data differ). Batches are
sorted by cost and dealt round-robin so each slot's max (rows, L) is
tight; the program only touches A[:, :ceil(F/128)*128, :Lpad] per slot.
"""

import numpy as np
import ml_dtypes

import concourse.bass as bass  # noqa: F401
import concourse.tile as tile
from concourse import bacc, mybir
from concourse.bass_utils import run_bass_kernel_spmd

B, T_DEC, T_ENC = 64, 2048, 512
G_STEPS, GAMMA = 20000, 0.99995
N_CORES = 8
SLOTS = B // N_CORES
GRP = 8  # 128-row chunks per dma_start (~1 MiB bf16 per transfer)

BF16 = ml_dtypes.bfloat16


def _fit_exp_poly(zmax: float) -> np.ndarray:
    """Monomial coefficients a_k with exp(z) ~= sum a_k z^k on [0, zmax]."""
    from numpy.polynomial import chebyshev as C

    zs = np.linspace(0.0, zmax, 4001)
    ez = np.exp(zs)
    for deg in range(8, 31, 2):
        a = C.cheb2poly(C.chebfit(zs, ez, deg))
        err = np.max(np.abs(np.polynomial.polynomial.polyval(zs, a) - ez))
        if err < 1e-9 * np.exp(zmax):
            return a
    return a


def _plan(input_lengths: np.ndarray, target_lengths: np.ndarray):
    """Assign 64 batches to 8 slots x 8 cores, minimizing per-slot max work.

    Tries two sort keys and keeps whichever yields fewer total bytes.
    (Even free dim is an ISA requirement for the matmul moving operand;
    pad L to a multiple of 4.)
    """
    F = target_lengths.astype(np.int64)
    L = input_lengths.astype(np.int64)
    chunks = (F + 127) // 128

    Lp = -4 * (-L // 4)

    def mk(order):
        sb = np.stack([order[i * N_CORES:(i + 1) * N_CORES]
                       for i in range(SLOTS)])
        return cost(sb), sb

    def cost(sb):
        return int((chunks[sb].max(1) * Lp[sb].max(1)).sum())

    cand1 = mk(np.argsort(-(chunks * L), kind="stable"))
    cand2 = mk(np.lexsort((-L, -chunks)))  # chunks primary, L secondary
    best, assign = min(cand1, cand2, key=lambda t: t[0])
    sb = [assign[i] for i in range(SLOTS)]
    sc = [int(chunks[s].max()) for s in sb]
    sl = [min(T_ENC, -4 * (-int(L[s].max()) // 4)) for s in sb]
    return sb, sc, sl


def _build_program(slot_chunks, slot_L, M):
    f32 = mybir.dt.float32
    bf = mybir.dt.bfloat16
    total_chunks = sum(slot_chunks)
    offs = np.concatenate([[0], np.cumsum(slot_chunks)]).astype(int)

    nc = bacc.Bacc(
        "TRN2", target_bir_lowering=False, debug=False, num_devices=N_CORES
    )
    a_dr = [
        nc.dram_tensor(f"a{i}", [slot_chunks[i] * 128, slot_L[i]], bf,
                       kind="ExternalInput")
        for i in range(SLOTS)
    ]
    h_dr = nc.dram_tensor("h", [128, total_chunks, M], bf,
                          kind="ExternalInput")
    c_dr = [
        nc.dram_tensor(f"c{i}", [M, slot_L[i]], f32, kind="ExternalOutput")
        for i in range(SLOTS)
    ]

    with tile.TileContext(nc) as tc:
        with (
            tc.tile_pool(name="hp", bufs=1) as hpool,
            tc.tile_pool(name="ap", bufs=6) as apool,
            tc.tile_pool(name="op", bufs=2) as opool,
            tc.tile_pool(name="pp", bufs=2, space="PSUM") as pspool,
        ):
            ht = hpool.tile([128, total_chunks, M], bf)
            nc.gpsimd.dma_start(ht[:, :, :], h_dr[:, :, :])
            for i in range(SLOTS):
                nch = slot_chunks[i]
                Lm = slot_L[i]
                ps = pspool.tile([M, Lm], f32, tag="ps")
                bounds = list(range(0, nch, GRP)) + [nch]
                for g0, g1 in zip(bounds, bounds[1:]):
                    gn = g1 - g0
                    at = apool.tile([128, GRP, 512], bf, tag="a")
                    src = a_dr[i][g0 * 128:g1 * 128, :].rearrange(
                        "(g p) l -> p g l", p=128
                    )
                    nc.sync.dma_start(at[:, :gn, :Lm], src)
                    for k in range(gn):
                        ch = g0 + k
                        nc.tensor.matmul(
                            ps[:, :],
                            ht[:, offs[i] + ch, :],
                            at[:, k, :Lm],
                            start=(ch == 0),
                            stop=(ch == nch - 1),
                        )
                ot = opool.tile([M, Lm], f32, tag="o")
                nc.scalar.copy(ot[:, :], ps[:, :])
                nc.gpsimd.dma_start(c_dr[i][:, :], ot[:, :])
    nc.compile()
    return nc


def _kernel_impl(alignments, input_lengths, target_lengths, global_step,
                 trace=False):
    step = int(global_step)
    if G_STEPS < step:
        return np.zeros((), dtype=np.float32), None

    g = GAMMA ** step
    c = 1.0 / (2.0 * g * g)
    a_poly = _fit_exp_poly(2.0 * c)
    D = len(a_poly) - 1
    # weight columns: [hi_0..hi_D, ones, lo_0..lo_D]
    M = 2 * (D + 1) + 1

    F = target_lengths.astype(np.int64)
    L = input_lengths.astype(np.int64)
    slot_batches, slot_chunks, slot_L = _plan(input_lengths, target_lengths)
    offs = np.concatenate([[0], np.cumsum(slot_chunks)]).astype(int)
    total_chunks = int(offs[-1])

    nc = _build_program(slot_chunks, slot_L, M)

    al = np.asarray(alignments, dtype=np.float32)
    in_maps = []
    for j in range(N_CORES):
        im = {}
        h_all = np.zeros((total_chunks, 128, M), dtype=BF16)
        for i in range(SLOTS):
            b = int(slot_batches[i][j])
            R = slot_chunks[i] * 128
            Lm = slot_L[i]
            im[f"a{i}"] = al[b, :R, :Lm].astype(BF16)
            Fb = int(F[b])
            y = np.arange(R, dtype=np.float64) / Fb
            h = np.zeros((R, D + 2), dtype=np.float64)
            for k in range(D + 1):
                h[:, k] = a_poly[k] * (2.0 * c * y) ** k * np.exp(-c * y * y)
            h[:, D + 1] = 1.0
            h[Fb:, :] = 0.0
            hi = h.astype(BF16)
            lo = (h - hi.astype(np.float64)).astype(BF16)
            blk = h_all[offs[i]:offs[i + 1]].reshape(R, M)
            blk[:, :D + 2] = hi
            blk[:, D + 2:] = lo[:, :D + 1]
        im["h"] = np.ascontiguousarray(h_all.transpose(1, 0, 2))
        in_maps.append(im)

    res = run_bass_kernel_spmd(nc, in_maps, list(range(N_CORES)), trace=trace)

    # Host epilogue: tiny [M, L] combinations per batch, f64.
    per_sample = np.zeros(B, dtype=np.float64)
    for j in range(N_CORES):
        for i in range(SLOTS):
            b = int(slot_batches[i][j])
            Lb = int(L[b])
            Cm = res.results[j][f"c{i}"].astype(np.float64)
            Ck = Cm[:D + 1, :Lb] + Cm[D + 2:, :Lb]  # hi + lo
            x = np.arange(Lb, dtype=np.float64) / Lb
            ex = np.exp(-c * x * x)
            gsum = np.zeros(Lb)
            xk = np.ones(Lb)
            for k in range(D + 1):
                gsum += Ck[k] * xk
                xk *= x
            per_sample[b] = Cm[D + 1, :Lb].sum() - (gsum * ex).sum()
    loss = np.float64(np.mean(per_sample / F.astype(np.float64)))
    return np.asarray(loss, dtype=np.float32), res


def kernel(alignments, input_lengths, target_lengths, global_step):
    loss, _ = _kernel_impl(alignments, input_lengths, target_lengths,
                           global_step)
    return loss



# revision 2
# speedup vs baseline: 1.0092x; 1.0092x over previous
"""GuidedAttentionLoss on 8 Trainium2 NeuronCores — fp8 streaming version.

Math: loss = mean_b( sum_{f<F_b, l<L_b} A[b,f,l] * w[b,f,l] / F_b ),
      w = 1 - exp(-c*(l/L - f/F)^2),  c = 1/(2*gamma^(2*step)).

exp(-c(x-y)^2) = exp(-cx^2)*exp(-cy^2)*exp(2cxy); exp(z) on [0, 2c] is a
degree-D polynomial, so the weight separates: e[f,l] = sum_k h_k[f]*g_k[l],
  h_k[f] = a_k*(2c*y)^k*exp(-c*y^2),  y = f/F
  g_k[l] = x^k*exp(-c*x^2),           x = l/L.
The device contracts over f: C[k,l] = sum_f h_k[f]*A[f,l] — a tiny-M matmul
streamed through the TensorEngine.

This version targets the HBM roofline:
- A is staged as fp8 e4m3 (element rounding is unbiased; averages out over
  the contraction — measured ~3e-4 on the final loss). Host pre-transposes
  each batch to [128, chunks*W] so every partition's DMA run is multi-KB.
- h is split 3-way into fp8 columns (h = h0/s0 + h1/s1 + h2/s2 with pow2
  per-column scales), recovering ~2^-12 relative weight precision. With
  poly degree D=8 that is M = 3*(D+1)+1 = 28 stationary columns.
- Matmuls use 3x column-group tiling (tile_position=(0,32j)): three chunks
  stream concurrently through disjoint 32-column strips of the PE array,
  accumulating into three separate PSUM banks — 3x TensorE throughput, so
  the PE stays off the critical path.
- A second tiny matmul (bf16, pow2-exact scales) folds the 3 col-groups and
  3 splits back into C[k,l]; a fused DVE multiply-reduce against the
  host-staged g_k[l] (mask and sign baked in) leaves 10 floats per slot, so
  output DMA is negligible.

Sharding: pure data parallel over batch. 64 batches -> 8 slots x 8 cores
(SPMD). Slot shapes (max chunks x max padded L over its 8 members) are
chosen by simulated annealing to minimize total transferred bytes.
"""

import numpy as np
import ml_dtypes

import concourse.bass as bass  # noqa: F401
import concourse.tile as tile
from concourse import bacc, mybir
from concourse.bass_utils import run_bass_kernel_spmd

B, T_DEC, T_ENC = 64, 2048, 512
G_STEPS, GAMMA = 20000, 0.99995
N_CORES = 8
SLOTS = B // N_CORES
NCOL = 3  # column-group tiling factor

FP8 = ml_dtypes.float8_e4m3  # TRN FP8_EXP4-compatible (max 240)
BF16 = ml_dtypes.bfloat16


def _fit_exp_poly(zmax: float):
    """Monomial coeffs a_k with exp(z) ~= sum a_k z^k on [0, zmax], deg<=9."""
    from numpy.polynomial import chebyshev as C

    zs = np.linspace(0.0, zmax, 4001)
    ez = np.exp(zs)
    for deg in range(4, 10):
        a = C.cheb2poly(C.chebfit(zs, ez, deg))
        err = np.max(np.abs(np.polynomial.polynomial.polyval(zs, a) - ez))
        if err < 3e-7 * np.exp(zmax):
            break
    return a


def _plan(input_lengths, target_lengths, iters=60000):
    """Assign 64 batches to 8 slots x 8 cores minimizing sum over slots of
    max(chunks)*max(Lpad) — the bytes every core transfers."""
    rng = np.random.default_rng(0)
    F = target_lengths.astype(np.int64)
    L = input_lengths.astype(np.int64)
    chunks = ((F + 127) // 128).astype(int)
    Lp = (-4 * (-L // 4)).astype(int)

    def cost(a):
        return sum(int(chunks[s].max() * Lp[s].max()) for s in a)

    order = np.lexsort((-Lp, -chunks))
    cur = [order[i * 8:(i + 1) * 8].copy() for i in range(8)]
    curc = cost(cur)
    best, bestA = curc, [s.copy() for s in cur]
    Tmax, Tmin = 2000.0, 1.0
    for it in range(iters):
        T = Tmax * (Tmin / Tmax) ** (it / iters)
        i1, i2 = rng.integers(0, 8, 2)
        if i1 == i2:
            continue
        j1, j2 = rng.integers(0, 8, 2)
        cur[i1][j1], cur[i2][j2] = cur[i2][j2], cur[i1][j1]
        cc = cost(cur)
        if cc < curc or rng.random() < np.exp((curc - cc) / T):
            curc = cc
            if cc < best:
                best, bestA = cc, [s.copy() for s in cur]
        else:
            cur[i1][j1], cur[i2][j2] = cur[i2][j2], cur[i1][j1]
    sb = bestA
    sc = [int(chunks[s].max()) for s in sb]
    sw = [int(Lp[s].max()) for s in sb]
    # smallest slot first (quick pipeline start), then big ones, ending on
    # the second-smallest (short post-DMA tail chain)
    order = list(np.argsort([-c * w for c, w in zip(sc, sw)], kind="stable"))
    order = order[-1:] + order[:-1]
    sb = [sb[i] for i in order]
    sc = [sc[i] for i in order]
    sw = [sw[i] for i in order]
    return sb, sc, sw


def _build_program(slot_chunks, slot_W, M, D2):
    f32 = mybir.dt.float32
    bf = mybir.dt.bfloat16
    f8 = mybir.dt.float8e4
    TC = sum(slot_chunks)
    TOT = sum(c * w for c, w in zip(slot_chunks, slot_W))
    CWMAX = max(c * w for c, w in zip(slot_chunks, slot_W))
    hoffs = np.concatenate([[0], np.cumsum(slot_chunks)]).astype(int)
    aoffs = np.concatenate(
        [[0], np.cumsum([c * w for c, w in zip(slot_chunks, slot_W)])]
    ).astype(int)

    nc = bacc.Bacc(
        "TRN2", target_bir_lowering=False, debug=False, num_devices=N_CORES
    )
    goffs = np.concatenate([[0], np.cumsum(slot_W)]).astype(int)
    a_dr = nc.dram_tensor("a", [128, TOT], f8, kind="ExternalInput")
    h_dr = nc.dram_tensor("h", [128, TC, M], f8, kind="ExternalInput")
    s_dr = nc.dram_tensor("s", [96, D2], bf, kind="ExternalInput")
    g_dr = nc.dram_tensor("g", [D2, int(goffs[-1])], bf,
                          kind="ExternalInput")
    r_dr = nc.dram_tensor("r", [D2, SLOTS], f32, kind="ExternalOutput")

    with tile.TileContext(nc) as tc:
        with (
            tc.tile_pool(name="hp", bufs=1) as hpool,
            tc.tile_pool(name="ap", bufs=4) as apool,
            tc.tile_pool(name="op", bufs=3) as opool,
            tc.tile_pool(name="gp", bufs=4) as gpool,
            tc.tile_pool(name="pp", bufs=2, space="PSUM") as pspool,
        ):
            st = hpool.tile([96, D2], bf)
            nc.gpsimd.dma_start(st[:, :], s_dr[:, :])
            # per-slot h TILES: slot i's matmuls gate only on their own
            # (tiny) h DMA, so the pipeline starts ~3us earlier
            hts = []
            for i in range(SLOTS):
                lo, hi = int(hoffs[i]), int(hoffs[i + 1])
                hti = hpool.tile([128, slot_chunks[i], M], f8, tag=f"h{i}")
                nc.gpsimd.dma_start(hti[:, :, :], h_dr[:, lo:hi, :])
                hts.append(hti)
            for i in range(SLOTS):
                C = slot_chunks[i]
                W = slot_W[i]
                at = apool.tile([128, CWMAX], f8, tag="a")
                # two sub-DMAs per slot: finer completion granularity lets
                # the first matmuls start earlier and buffers recycle sooner
                h1 = (C // 2) * W
                nc.sync.dma_start(
                    at[:, :h1], a_dr[:, int(aoffs[i]):int(aoffs[i]) + h1]
                )
                nc.sync.dma_start(
                    at[:, h1:C * W],
                    a_dr[:, int(aoffs[i]) + h1:int(aoffs[i + 1])],
                )
                gt = gpool.tile([D2, 512], bf, tag="g")
                nc.scalar.dma_start(
                    gt[:, :W], g_dr[:, int(goffs[i]):int(goffs[i + 1])]
                )
                ps = pspool.tile([128, NCOL * 512], f32, tag="ps")
                for q in range(C):
                    j = q % NCOL
                    nc.tensor.matmul(
                        ps[32 * j:32 * j + M, 512 * j:512 * j + W],
                        hts[i][:, q, :],
                        at[:, q * W:(q + 1) * W],
                        start=(q < NCOL),
                        stop=(q + NCOL >= C),
                        tile_position=(0, 32 * j),
                    )
                ot = opool.tile([96, 512], bf, tag="o")
                nc.vector.tensor_copy(ot[0:32, :W], ps[0:32, 0:W])
                nc.scalar.copy(ot[32:64, :W], ps[32:64, 512:512 + W])
                nc.scalar.copy(ot[64:96, :W], ps[64:96, 1024:1024 + W])
                pc = pspool.tile([D2, 512], f32, tag="pc")
                nc.tensor.matmul(
                    pc[:, :W], st[:, :], ot[:, :W], start=True, stop=True
                )
                tmp = opool.tile([D2, 512], f32, tag="t")
                rt = opool.tile([D2, 1], f32, tag="r")
                nc.vector.tensor_tensor(
                    tmp[:, :W], pc[:, :W], gt[:, :W], op=mybir.AluOpType.mult
                )
                nc.vector.reduce_sum(
                    rt[:, :], tmp[:, :W], axis=mybir.AxisListType.X
                )
                nc.scalar.dma_start(r_dr[:, i:i + 1], rt[:, :])
    nc.compile()
    return nc


def _kernel_impl(alignments, input_lengths, target_lengths, global_step,
                 trace=False):
    step = int(global_step)
    if G_STEPS < step:
        return np.zeros((), dtype=np.float32), None

    g = GAMMA ** step
    c = 1.0 / (2.0 * g * g)
    a_poly = _fit_exp_poly(2.0 * c)
    D = len(a_poly) - 1
    D2 = D + 2
    # [split0 k=0..D | split1 | split2 | ones | zero-pad to 32] — full 32
    # columns so every PSUM stripe row is matmul-written (no garbage reads).
    M = 3 * (D + 1) + 1
    assert M <= 32
    MP = 32

    F = target_lengths.astype(np.int64)
    L = input_lengths.astype(np.int64)
    slot_batches, slot_chunks, slot_W = _plan(input_lengths, target_lengths)
    hoffs = np.concatenate([[0], np.cumsum(slot_chunks)]).astype(int)
    TC = int(hoffs[-1])

    # global per-column pow2 scales (max |h_k| scaled to ~16)
    yg = np.linspace(0, 1.13, 2000)
    sig = np.zeros((3, D + 1))
    for k in range(D + 1):
        mx = np.max(np.abs(a_poly[k] * (2.0 * c * yg) ** k * np.exp(-c * yg * yg)))
        sig[0, k] = 2.0 ** np.floor(np.log2(16.0 / mx)) if mx > 0 else 1.0
    sig[1] = sig[0] * 16.0
    sig[2] = sig[0] * 256.0

    Scomb = np.zeros((96, D2), dtype=BF16)
    for s in range(3):
        for k in range(D + 1):
            for jcol in range(NCOL):
                Scomb[32 * jcol + s * (D + 1) + k, k] = BF16(1.0 / sig[s, k])
    for jcol in range(NCOL):
        Scomb[32 * jcol + M - 1, D + 1] = BF16(1.0)

    nc = _build_program(slot_chunks, slot_W, MP, D2)

    al = np.asarray(alignments, dtype=np.float32)
    goffs = np.concatenate([[0], np.cumsum(slot_W)]).astype(int)
    in_maps = []
    for j in range(N_CORES):
        im = {"s": Scomb}
        a_parts = []
        h_all = np.zeros((TC, 128, MP), dtype=FP8)
        g_all = np.zeros((D2, int(goffs[-1])), dtype=BF16)
        for i in range(SLOTS):
            b = int(slot_batches[i][j])
            C, W = slot_chunks[i], slot_W[i]
            R = C * 128
            blk = al[b, :R, :W].astype(FP8)
            a_parts.append(
                blk.reshape(C, 128, W).transpose(1, 0, 2).reshape(128, C * W)
            )
            Fb, Lb = int(F[b]), int(L[b])
            y = np.arange(R, dtype=np.float64) / Fb
            h = np.zeros((R, D + 1), dtype=np.float64)
            for k in range(D + 1):
                h[:, k] = a_poly[k] * (2.0 * c * y) ** k * np.exp(-c * y * y)
            h[Fb:, :] = 0.0
            hb = np.zeros((R, MP), dtype=FP8)
            resid = h
            for s in range(3):
                qs = (resid * sig[s][None, :]).astype(FP8)
                hb[:, s * (D + 1):(s + 1) * (D + 1)] = qs
                resid = resid - qs.astype(np.float64) / sig[s][None, :]
            ones = np.zeros(R, dtype=np.float64)
            ones[:Fb] = 1.0
            hb[:, M - 1] = ones.astype(FP8)
            h_all[int(hoffs[i]):int(hoffs[i + 1])] = (
                hb.reshape(C, 128, MP)
            )
            # g: mask + sign baked; row D+1 is the ones row (+1)
            x = np.arange(W, dtype=np.float64) / Lb
            ex = np.exp(-c * x * x)
            gt = np.zeros((D2, W), dtype=np.float64)
            for k in range(D + 1):
                gt[k] = -(x ** k) * ex
            gt[D + 1] = 1.0
            gt[:, Lb:] = 0.0
            g_all[:, int(goffs[i]):int(goffs[i + 1])] = gt.astype(BF16)
        im["a"] = np.ascontiguousarray(np.concatenate(a_parts, axis=1))
        im["h"] = np.ascontiguousarray(h_all.transpose(1, 0, 2))
        im["g"] = g_all
        in_maps.append(im)

    res = run_bass_kernel_spmd(nc, in_maps, list(range(N_CORES)), trace=trace)

    per_sample = np.zeros(B, dtype=np.float64)
    for j in range(N_CORES):
        rj = res.results[j]["r"].astype(np.float64)
        for i in range(SLOTS):
            b = int(slot_batches[i][j])
            per_sample[b] += float(rj[:, i].sum())
    loss = np.float64(np.mean(per_sample / F.astype(np.float64)))
    return np.asarray(loss, dtype=np.float32), res


def kernel(alignments, input_lengths, target_lengths, global_step):
    loss, _ = _kernel_impl(alignments, input_lengths, target_lengths,
                           global_step)
    return loss
